# revision 8
# baseline (speedup 1.0000x reference)
"""Distributed Trainium2 Bass kernel for the associative-embedding (AE) loss.

Problem: per image b (B=8), two tag maps (tm0 [J,256,256], tm1 [J,512,512]),
keypoints kps [NH, 3*J] (x, y, vis interleaved, NH=30 humans, J=17 joints).
Per level: gather tag values at (j, x, y), masked per-human mean, pull loss
(masked squared deviation / num_humans) + push loss (pairwise Gaussian of
means / num_humans^2).  Output: per-image loss [B] (sum over both levels).

Strategy: pure data-parallel over B across 8 NeuronCores (core b handles
image b).  The loss touches only NH*J = 510 elements of each tag map, so
instead of streaming the 178 MB of tag maps, each core computes flat gather
indices on-chip from the keypoint data and pulls exactly 1020 scalars out
of DRAM via 8 indirect (SWDGE) DMAs of 128 single-element descriptors each
(HW indirect DMA = one descriptor per out partition).  The gathered values
live in a [128, 8] "chunk" layout; one-hot matrices passed from the host
let the tensor engine reduce that layout directly into the per-human
sufficient statistics (sum of masked vals, sum of masked vals^2) since
pull = sum(m v^2) - sv*avg.  The push loss uses a 32x32 DVE stream
transpose for the pairwise mean differences.  Per-core output is a single
scalar; the host stacks the 8 scalars into the final [8] vector.
"""

import numpy as np

B = 8
NH = 30
J = 17
H0 = W0 = 256
H1 = W1 = 512
N0 = J * H0 * W0
N1 = J * H1 * W1
NTOT = N0 + N1
NR = 2 * J * NH           # 1020 gathered elements
NC = 8                    # gather chunks of 128
BIG = 1.0e9               # pad avg rows 30/31 -> exp(-BIG^2/2) = 0

_CACHE = {}

# ---------------------------------------------------------------------------
# host-side constant layouts for the [128, 8] gather chunk layout
# r = c*128 + p encodes (f, nh): f = r // NH (l*J + j), nh = r % NH
# ---------------------------------------------------------------------------


def _host_constants():
    if "consts" in _CACHE:
        return _CACHE["consts"]
    r = np.arange(128 * NC)
    valid = r < NR
    f = np.where(valid, r // NH, 0)
    nh = np.where(valid, r % NH, 0)
    lvl = f // J
    j = f % J

    # ki int32 [128, 40]: cols 0:24 = (x,y,vis) per chunk (filled per batch),
    # cols 24:32 = W multiplier, cols 32:40 = level/joint base offset.
    wmul = np.where(valid, np.where(lvl == 0, W0, W1), 0)
    base = np.where(valid, np.where(lvl == 0, j * H0 * W0, N0 + j * H1 * W1), 0)
    kcw = wmul.reshape(NC, 128).T.astype(np.int32)
    kcb = base.reshape(NC, 128).T.astype(np.int32)

    # cf f32 [128, 16 + 240]: cols 0:8 L0 indicator, 8:16 L1 indicator,
    # 16:256 E (one-hot of nh) per chunk.
    L0 = (valid & (lvl == 0)).astype(np.float32).reshape(NC, 128).T
    L1 = (valid & (lvl == 1)).astype(np.float32).reshape(NC, 128).T
    E = np.zeros((128, NC * NH), dtype=np.float32)
    for c in range(NC):
        rr = np.arange(c * 128, (c + 1) * 128)
        ok = rr < NR
        E[ok, c * NH + (rr[ok] % NH)] = 1.0
    cf = np.concatenate([L0, L1, E], axis=1).astype(np.float32)

    # per-batch gather-layout keypoint index helper
    # x(r) = kp_cat[nh, lvl*51 + 3j], sim. y, vis
    col_x = (lvl * 3 * J + 3 * j).astype(np.int64)
    _CACHE["consts"] = dict(
        kcw=kcw, kcb=kcb, cf=cf, nh=nh, col_x=col_x, valid=valid
    )
    return _CACHE["consts"]


def make_in_maps(tag_maps0, tag_maps1, kps0, kps1):
    tag_maps0 = np.asarray(tag_maps0, dtype=np.float32)
    tag_maps1 = np.asarray(tag_maps1, dtype=np.float32)
    kps0 = np.asarray(kps0, dtype=np.int32)
    kps1 = np.asarray(kps1, dtype=np.int32)
    C = _host_constants()
    nh, col_x, valid = C["nh"], C["col_x"], C["valid"]
    in_maps = []
    for b in range(B):
        tm = np.concatenate(
            [tag_maps0[b].ravel(), tag_maps1[b].ravel()]
        ).reshape(NTOT, 1)
        kp = np.concatenate([kps0[b], kps1[b]], axis=1)  # [30, 102]
        # gather-layout (x, y, vis): pure relayout of kp
        kpg = np.zeros((128 * NC, 3), dtype=np.int32)
        kpg[valid, 0] = kp[nh[valid], col_x[valid]]
        kpg[valid, 1] = kp[nh[valid], col_x[valid] + 1]
        kpg[valid, 2] = kp[nh[valid], col_x[valid] + 2]
        kpg = kpg.reshape(NC, 128, 3).transpose(1, 0, 2).reshape(128, NC * 3)
        ki = np.concatenate([kpg, C["kcw"], C["kcb"]], axis=1)  # [128, 40]
        in_maps.append({"tm": tm, "kp": kp, "ki": ki, "cf": C["cf"]})
    return in_maps


# ---------------------------------------------------------------------------
# device kernel
# ---------------------------------------------------------------------------


def _build_nc():
    from concourse import bacc, mybir
    import concourse.tile as tile
    from concourse.bass import IndirectOffsetOnAxis

    f32 = mybir.dt.float32
    i32 = mybir.dt.int32
    Alu = mybir.AluOpType
    X = mybir.AxisListType.X

    nc = bacc.Bacc()
    TM = nc.declare_dram_parameter("tm", [NTOT, 1], f32, isOutput=False)
    KP = nc.declare_dram_parameter("kp", [NH, 6 * J], i32, isOutput=False)
    KI = nc.declare_dram_parameter("ki", [128, 3 * NC + 2 * NC], i32, isOutput=False)
    CF = nc.declare_dram_parameter("cf", [128, 2 * NC + NC * NH], f32, isOutput=False)
    OUT = nc.declare_dram_parameter("out", [1, 1], f32, isOutput=True)

    with tile.TileContext(nc) as tc:
        with (
            tc.tile_pool(name="sb", bufs=1) as sb,
            tc.tile_pool(name="pp", bufs=1, space="PSUM") as pp,
        ):
            kt = sb.tile([NH, 6 * J], i32)
            ki = sb.tile([128, 5 * NC], i32)
            cf = sb.tile([128, 2 * NC + NC * NH], f32)
            idxg = sb.tile([128, NC], i32)
            S = sb.tile([128, NC], f32)
            maskg = sb.tile([128, NC], f32)
            Sg = sb.tile([128, NC], f32)
            Sg2 = sb.tile([128, NC], f32)
            T = sb.tile([128, 4 * NC], f32)
            maskf = sb.tile([NH, 2 * J], f32)
            cnt = sb.tile([NH, 2], f32)
            den = sb.tile([NH, 2], f32)
            rden = sb.tile([NH, 2], f32)
            st = sb.tile([NH, 4], f32)
            avg0 = sb.tile([NH, 2], f32)
            u = sb.tile([NH, 2], f32)
            avg32 = sb.tile([32, 2], f32)
            avgsrc = sb.tile([32, 64], f32)
            avgT = sb.tile([32, 64], f32)
            d2 = sb.tile([NH, 64], f32)
            pm = sb.tile([NH, 64], f32)
            pack = sb.tile([NH, 6], f32)
            ones = sb.tile([NH, 1], f32)
            warm = sb.tile([1, 1], f32)
            sums = sb.tile([1, 6], f32)
            rec = sb.tile([1, 6], f32)
            m1 = sb.tile([1, 4], f32)
            res = sb.tile([1, 1], f32)
            ps_st = pp.tile([NH, 4], f32)
            ps_f = pp.tile([1, 6], f32)

            # Warm the ACT Exp table while DMAs run.
            nc.vector.memset(warm[:], 0.0)
            nc.scalar.activation(
                warm[:], warm[:], mybir.ActivationFunctionType.Exp
            )

            # Inputs in.
            nc.sync.dma_start(ki[:], KI[:])
            nc.sync.dma_start(kt[:], KP[:])
            nc.sync.dma_start(cf[:], CF[:])

            # Gather indices in chunk layout: idx = x*W + y + base.
            xg = ki[:, 0 : 3 * NC : 3]
            yg = ki[:, 1 : 3 * NC : 3]
            vg = ki[:, 2 : 3 * NC : 3]
            wm = ki[:, 3 * NC : 4 * NC]
            bs = ki[:, 4 * NC : 5 * NC]
            nc.vector.tensor_tensor(out=idxg[:], in0=xg, in1=wm, op=Alu.mult)
            nc.vector.tensor_tensor(out=idxg[:], in0=idxg[:], in1=yg, op=Alu.add)
            nc.vector.tensor_tensor(out=idxg[:], in0=idxg[:], in1=bs, op=Alu.add)

            # The only touch of the big tag maps: 1020 gathered scalars in
            # 8 indirect DMAs (128 single-element descriptors each).
            for c in range(NC):
                nc.gpsimd.indirect_dma_start(
                    out=S[:, c : c + 1],
                    out_offset=None,
                    in_=TM[:],
                    in_offset=IndirectOffsetOnAxis(ap=idxg[:, c : c + 1], axis=0),
                )

            # Masked first/second moments in chunk layout.
            nc.vector.tensor_scalar(
                out=maskg[:], in0=vg, scalar1=0, scalar2=None, op0=Alu.is_gt
            )
            nc.vector.tensor_tensor(out=Sg[:], in0=S[:], in1=maskg[:], op=Alu.mult)
            nc.vector.tensor_tensor(out=Sg2[:], in0=Sg[:], in1=S[:], op=Alu.mult)
            L0 = cf[:, 0:NC]
            L1 = cf[:, NC : 2 * NC]
            nc.vector.tensor_tensor(out=T[:, 0:NC], in0=Sg[:], in1=L0, op=Alu.mult)
            nc.vector.tensor_tensor(
                out=T[:, NC : 2 * NC], in0=Sg[:], in1=L1, op=Alu.mult
            )
            nc.vector.tensor_tensor(
                out=T[:, 2 * NC : 3 * NC], in0=Sg2[:], in1=L0, op=Alu.mult
            )
            nc.vector.tensor_tensor(
                out=T[:, 3 * NC : 4 * NC], in0=Sg2[:], in1=L1, op=Alu.mult
            )

            # Per-human stats via one-hot matmuls: st = [sv0, sv1, s2_0, s2_1].
            for c in range(NC):
                nc.tensor.matmul(
                    ps_st[:],
                    lhsT=cf[:, 2 * NC + c * NH : 2 * NC + (c + 1) * NH],
                    rhs=T[:, c : 4 * NC : NC],
                    start=(c == 0),
                    stop=(c == NC - 1),
                )
            nc.vector.tensor_copy(out=st[:], in_=ps_st[:])

            # Joint counts from the original keypoint layout.
            vis = kt[:, 2 : 6 * J : 3]
            nc.vector.tensor_scalar(
                out=maskf[:], in0=vis, scalar1=0, scalar2=None, op0=Alu.is_gt
            )
            nc.vector.reduce_sum(
                out=cnt[:], in_=maskf[:].rearrange("p (l j) -> p l j", l=2), axis=X
            )
            nc.vector.tensor_scalar(
                out=den[:], in0=cnt[:], scalar1=1.0, scalar2=None, op0=Alu.max
            )
            nc.vector.reciprocal(rden[:], den[:])
            sv = st[:, 0:2]
            s2 = st[:, 2:4]
            nc.vector.tensor_tensor(out=avg0[:], in0=sv, in1=rden[:], op=Alu.mult)
            nc.vector.tensor_scalar(
                out=pack[:, 4:6], in0=cnt[:], scalar1=0.0, scalar2=None, op0=Alu.is_gt
            )
            nc.vector.memset(avg32[:], BIG)
            nc.vector.tensor_tensor(
                out=avg32[0:NH, :], in0=avg0[:], in1=pack[:, 4:6], op=Alu.mult
            )
            # pull = s2 - sv*avg0 (zero when cnt == 0 since sv = s2 = 0)
            nc.vector.tensor_tensor(out=u[:], in0=sv, in1=avg0[:], op=Alu.mult)
            nc.vector.tensor_tensor(
                out=pack[:, 0:2], in0=s2, in1=u[:], op=Alu.subtract
            )

            # Push: pairwise means via 32x32 block stream transpose.
            nc.vector.tensor_copy(
                out=avgsrc[:].rearrange("p (l j) -> p l j", l=2),
                in_=avg32[:].to_broadcast([32, 2, 32]),
            )
            nc.vector.transpose(avgT[:], avgsrc[:])
            nc.vector.tensor_tensor(
                out=d2[:].rearrange("p (l j) -> p l j", l=2),
                in0=avgT[0:NH, :].rearrange("p (l j) -> p l j", l=2),
                in1=avg32[0:NH, :].to_broadcast([NH, 2, 32]),
                op=Alu.subtract,
            )
            nc.vector.tensor_tensor(out=d2[:], in0=d2[:], in1=d2[:], op=Alu.mult)
            nc.scalar.activation(
                pm[:], d2[:], mybir.ActivationFunctionType.Exp, scale=-0.5
            )
            nc.vector.reduce_sum(
                out=pack[:, 2:4],
                in_=pm[:].rearrange("p (l j) -> p l j", l=2),
                axis=X,
            )

            # Column sums over the 30 humans via PE, then the final scalar.
            nc.vector.memset(ones[:], 1.0)
            nc.tensor.matmul(
                ps_f[:], lhsT=ones[:], rhs=pack[:], start=True, stop=True
            )
            nc.vector.tensor_copy(out=sums[:], in_=ps_f[:])
            nc.vector.reciprocal(rec[:], sums[:])
            nc.vector.tensor_tensor(
                out=m1[:, 0:2], in0=sums[:, 0:2], in1=rec[:, 4:6], op=Alu.mult
            )
            nc.vector.tensor_tensor(
                out=m1[:, 2:4], in0=sums[:, 2:4], in1=rec[:, 4:6], op=Alu.mult
            )
            nc.vector.tensor_tensor(
                out=m1[:, 2:4], in0=m1[:, 2:4], in1=rec[:, 4:6], op=Alu.mult
            )
            nc.vector.reduce_sum(out=res[:], in_=m1[:], axis=X)

            nc.sync.dma_start(OUT[:], res[:])

    nc.finalize()
    return nc


def _get_nc():
    if "nc" not in _CACHE:
        _CACHE["nc"] = _build_nc()
    return _CACHE["nc"]


def kernel(tag_maps0, tag_maps1, kps0, kps1):
    from concourse.bass_utils import run_bass_kernel_spmd

    nc = _get_nc()
    in_maps = make_in_maps(tag_maps0, tag_maps1, kps0, kps1)
    out = run_bass_kernel_spmd(nc, in_maps, core_ids=list(range(B)))
    return np.array(
        [np.asarray(out.results[b]["out"]).reshape(()) for b in range(B)],
        dtype=np.float32,
    )
